# revision 65
# baseline (speedup 1.0000x reference)
"""Multi-head causal attention (B=4, S=2048, D=1024, H=16) on 8 Trainium2
NeuronCores via Bass/Tile — fused projection+attention, fp8 Q/K projections.

Sharding: core c handles batch b = c//2 and head-group g = c%2 (8 heads,
i.e. columns [512g, 512g+512) of Wq/Wk/Wv and rows [512g, 512g+512) of Wo).
Each core computes its 8 heads' attention and a partial output projection
[S, D] in fp16; the host sums the two head-group partials per batch + bo.

Fusion: the sq loop projects q/k/v for one 512-wide chunk, then runs that
chunk's attention (which only needs k/v history up to the chunk). This
keeps the ACT engine's exp stream (the attention phase's rate limiter)
running from early on instead of idling through a projection phase.

Numerics: Q/K projections run in fp8e4 DoubleRow (K=256 per instruction,
halving the chain); everything downstream incl. scores stays fp16 (fp8
scores don't speed up fixed-N matmuls on HW, and fp8 anywhere in the
value path — measured both v-tiles and the v-projection — fails the
2e-2 gate). Wq/Wk are pre-scaled x32 for e4m3's normal range; the x1024
score scale folds into the exp activation scale. Denominator rows ride
the 65th column of the V tiles (ones) through the AV matmuls; den rows
travel via tiny SBUF->SBUF DMAs to a per-chunk [8,512] tile, get the
fast fp32 reciprocal eagerly, and are partition-replicated by one PE
broadcast matmul per pair (selh2 one-hot lhsT). Normalization is
deferred into the next chunk's compute so the den-DMA -> reciprocal ->
broadcast chain never stalls the PE; the epilogue drains the previous
chunk's ready outprojs BEFORE the final normalize so its broadcast
matmuls don't block them in the PE's in-order queue. (A DMA-replicate
normalize without PE broadcasts was measured SLOWER: its dependency
stalls on the shared DMA queues back-pressure the PSUM->SBUF copy
chain and cost ~10-25us.)

DMA: loads are merged into one instruction per logical tensor chunk
(halved across the two HWDGE queues); wq/wk arrive host-pre-shuffled
pair-major so pair-0's 128KB lands first and its projection starts
~5us earlier; bulk wv/wo/mask/selh2 ride the otherwise-idle GpSimd
SWDGE queue (a third wire). The last chunk's den-row DMAs use the
by-then-idle ACT queue. A few dummy warmup matmuls on memset SBUF
bridge the HAM clock ramp while the first weights are on the wire.
The causal mask01 multiply runs on GpSimd, which carries no other
blocking work (its strict FIFO feeds the AV matmuls).

PSUM budget (8 banks): sc ring 2x[128,1024] (4) + av ring 2x[65,512] (2)
+ general ring 2x[128,512] (2) shared by projections/out-proj.
"""

import sys
import numpy as np

for _p in ("/opt/trn_rl_repo", "/root/.axon_site/_ro/trn_rl_repo"):
    if _p not in sys.path:
        sys.path.append(_p)

B, S_FULL, D, H, DK = 4, 2048, 1024, 16, 64
GD = 512          # dk span per core (8 heads)
P = 128
NPAIR = GD // P   # 4 head-pairs per core
N_CORES = 8
MASK_NEG = -8.0e9
WSCALE = 32.0     # Wq/Wk pre-scale for fp8
ISCALE = 1.0 / (WSCALE * WSCALE)

_BUILD_CACHE = {}


def _build(s_len):
    from contextlib import ExitStack

    import concourse.tile as tile
    from concourse import bacc, mybir

    dt = mybir.dt
    f32, f16, bf16, f8 = dt.float32, dt.float16, dt.bfloat16, dt.float8e4
    Exp = mybir.ActivationFunctionType.Exp
    DR = mybir.MatmulPerfMode.DoubleRow

    S = s_len
    SJ = S // 512     # 512-wide sq chunks
    SM = S // P       # 128-wide sk chunks
    DC = D // P       # fp16 contraction chunks
    DC2 = D // 256    # fp8 DoubleRow contraction steps

    nc = bacc.Bacc("TRN2", target_bir_lowering=False, debug=False,
                   num_devices=N_CORES)

    xq = nc.dram_tensor("xq", [D, S], f8, kind="ExternalInput")
    xk = nc.dram_tensor("xk", [D, S], f8, kind="ExternalInput")
    xv = nc.dram_tensor("xv", [D, S], f16, kind="ExternalInput")
    # wq/wk arrive host-pre-shuffled to [(pair p), (d t m)] so each
    # pair's weights are one contiguous 128KB DMA with 1KB descriptors
    # (pair-0's weights land ~10us in, letting its projection start
    # while the other pairs' weights are still on the wire)
    wq = nc.dram_tensor("wq", [NPAIR * P, DC2 * 2 * P], f8,
                        kind="ExternalInput")
    wk = nc.dram_tensor("wk", [NPAIR * P, DC2 * 2 * P], f8,
                        kind="ExternalInput")
    wv = nc.dram_tensor("wv", [D, GD], f16, kind="ExternalInput")
    wo = nc.dram_tensor("wo", [GD, D], f16, kind="ExternalInput")
    # mask01[p, c] = 0 where p > c else 1: zeroes the masked triangle
    # of each diagonal window on the exp output (on the idle GpSimd
    # engine, so the PE pays nothing for causal masking)
    mask01 = nc.dram_tensor("mask01", [P, P], f16, kind="ExternalInput")
    # selh2[:, 128p+64e:...+64] is one-hot row 2p+e: used ONLY for the
    # final chunk, where a PE broadcast matmul beats the DMA-replicate
    # path (the PE is idle in the tail and the DMA hops cost ~5us)
    selh2 = nc.dram_tensor("selh2", [8, 4 * 128], f16, kind="ExternalInput")
    out = nc.dram_tensor("out", [S, D], f16, kind="ExternalOutput")

    # merged-load DRAM views
    wv_v = wv.ap().rearrange("(d p) m -> p d m", p=P)
    wo_v = wo.ap().rearrange("(p r) m -> r p m", r=P)

    with tile.TileContext(nc) as tc, ExitStack() as ctx0:
        persist = ctx0.enter_context(tc.tile_pool(name="persist", bufs=1))

        qTt = [persist.tile([P, S], f16, tag=f"qT{p}", name=f"qT{p}")
               for p in range(NPAIR)]
        kTt = [persist.tile([P, S], f16, tag=f"kT{p}", name=f"kT{p}")
               for p in range(NPAIR)]
        vt = [persist.tile([P, 8 * 65], f16, tag=f"v{m}", name=f"v{m}")
              for m in range(SM)]
        ctxt = [persist.tile([P, S], f16, tag=f"ctx{p}", name=f"ctx{p}")
                for p in range(NPAIR)]
        mask01_t = persist.tile([P, P], f16, tag="mask01")
        selh2_t = persist.tile([8, 4 * 128], f16, tag="selh2")

        # weights, merged tiles
        wq8m = persist.tile([P, NPAIR * DC2 * 2 * P], f8, tag="wq",
                            name="wq")
        wk8m = persist.tile([P, NPAIR * DC2 * 2 * P], f8, tag="wk",
                            name="wk")
        wvm = persist.tile([P, DC * GD], f16, tag="wv", name="wv")
        wom = persist.tile([P, NPAIR * D], f16, tag="wo", name="wo")
        wq5 = wq8m[:].rearrange("p (i d t m) -> p i d t m", i=NPAIR,
                                d=DC2, t=2)
        wk5 = wk8m[:].rearrange("p (i d t m) -> p i d t m", i=NPAIR,
                                d=DC2, t=2)
        wv3 = wvm[:].rearrange("p (d m) -> p d m", d=DC)
        wo3 = wom[:].rearrange("p (q m) -> p q m", q=NPAIR)

        # HAM warmup scratch: the PE clock sits at 1.2 GHz until ~3.4us
        # of sustained matmul activity; dummy matmuls on memset SBUF
        # warm it to 2.4 GHz while the first weight DMAs are still on
        # the wire, so the real matmul stream runs at full clock
        dummy_w = persist.tile([P, 512], f16, tag="dumw", name="dumw")

        def load_w_pair(eng, wsb, wdram, i):
            eng.dma_start(wsb[:, i * DC2 * 2 * P:(i + 1) * DC2 * 2 * P],
                          wdram.ap()[i * P:(i + 1) * P, :])

        # pair-0 weights first on both HWDGE queues; wv rides the
        # otherwise-idle GpSimd SWDGE queue (3rd wire) so the chunk-0
        # AV isn't gated behind it; wo (needed only ~40us in) goes on
        # sync AFTER the critical chunk-0 loads
        load_w_pair(nc.sync, wq8m, wq, 0)
        load_w_pair(nc.scalar, wk8m, wk, 0)
        nc.gpsimd.dma_start(mask01_t[:], mask01.ap())
        nc.gpsimd.dma_start(wv3[:], wv_v)
        nc.gpsimd.dma_start(selh2_t[:], selh2.ap())

        with ExitStack() as ctx2:
            xpool = ctx2.enter_context(tc.tile_pool(name="xst", bufs=4))
            xvpool = ctx2.enter_context(tc.tile_pool(name="xvt", bufs=2))

            scps = ctx2.enter_context(
                tc.tile_pool(name="scps", bufs=2, space="PSUM"))
            avps = ctx2.enter_context(
                tc.tile_pool(name="avps", bufs=2, space="PSUM"))
            gpps = ctx2.enter_context(
                tc.tile_pool(name="gpps", bufs=2, space="PSUM"))
            expp = ctx2.enter_context(tc.tile_pool(name="expp", bufs=8))
            avsb = ctx2.enter_context(tc.tile_pool(name="avsb", bufs=12))
            rcp = ctx2.enter_context(tc.tile_pool(name="rcp", bufs=2))
            osb = ctx2.enter_context(tc.tile_pool(name="osb", bufs=4))

            def stage_x8(dram, sj, tag):
                """Merged fp8 x column-chunk load (2 DMAs: sync+scalar)."""
                c0, c1 = sj * 512, (sj + 1) * 512
                t = xpool.tile([P, DC2 * 2 * 512], f8, tag=tag, name=tag)
                t4 = t[:].rearrange("p (d t m) -> p d t m", d=DC2, t=2)
                src = dram.ap()[:, c0:c1].rearrange(
                    "(d t p) m -> p d t m", t=2, p=P)
                nc.sync.dma_start(t4[:, 0:2], src[:, 0:2])
                nc.scalar.dma_start(t4[:, 2:4], src[:, 2:4])
                return t4

            def stage_xv(sj):
                """Merged fp16 xv chunk load (2 DMAs: sync+scalar)."""
                c0, c1 = sj * 512, (sj + 1) * 512
                t = xvpool.tile([P, DC * 512], f16, tag="xv", name="xv")
                t3 = t[:].rearrange("p (d m) -> p d m", d=DC)
                src = xv.ap()[:, c0:c1].rearrange("(d p) m -> p d m", p=P)
                nc.sync.dma_start(t3[:, 0:4], src[:, 0:4])
                nc.scalar.dma_start(t3[:, 4:8], src[:, 4:8])
                return t3

            def stage(sj):
                return (stage_x8(xq, sj, "xq"), stage_x8(xk, sj, "xk"),
                        stage_xv(sj))

            def project_units(sj, staged):
                """q/k/v projection work for chunk sj as a list of psum-
                group closures; consumed one per m-block inside the
                PREVIOUS chunk's attention so the exp stream never
                pauses for a projection phase."""
                xq4, xk4, xv3_ = staged
                units = []

                def qk_unit(w5, x4, dstT, i):
                    ps = gpps.tile([P, 512], f32, tag="gp", name="gp")
                    for d in range(DC2):
                        nc.tensor.matmul(
                            ps[:],
                            w5[:, i, d],
                            x4[:, d],
                            start=(d == 0), stop=(d == DC2 - 1),
                            perf_mode=DR)
                    nc.vector.tensor_copy(
                        dstT[i][:, sj * 512:(sj + 1) * 512], ps[:])

                def v_unit(si):
                    ps = gpps.tile([P, 512], f32, tag="gp", name="gp")
                    off = (si - 4 * sj) * P
                    for d in range(DC):
                        nc.tensor.matmul(
                            ps[:],
                            xv3_[:, d, off:off + P],
                            wv3[:, d],
                            start=(d == 0), stop=(d == DC - 1))
                    v3 = vt[si][:].rearrange("p (h c) -> p h c", h=8)
                    nc.vector.tensor_copy(
                        v3[:, :, 0:64],
                        ps[:].rearrange("p (h c) -> p h c", h=8))
                    nc.vector.memset(v3[:, :, 64:65], 1.0)

                import functools
                if sj == 0:
                    # pair-0 q/k first (attention p=0 starts right after);
                    # then all v (p=0's AV drain needs them), then the
                    # remaining q/k pairs in p order
                    units.append(functools.partial(qk_unit, wq5, xq4,
                                                   qTt, 0))
                    units.append(functools.partial(qk_unit, wk5, xk4,
                                                   kTt, 0))
                    for si in range(4):
                        units.append(functools.partial(v_unit, si))
                    for i in range(1, NPAIR):
                        units.append(functools.partial(qk_unit, wq5, xq4,
                                                       qTt, i))
                        units.append(functools.partial(qk_unit, wk5, xk4,
                                                       kTt, i))
                else:
                    for w5_, x4_, dstT in ((wq5, xq4, qTt),
                                           (wk5, xk4, kTt)):
                        for i in range(NPAIR):
                            units.append(
                                functools.partial(qk_unit, w5_, x4_,
                                                  dstT, i))
                    for si in range(4 * sj, 4 * sj + 4):
                        units.append(functools.partial(v_unit, si))
                return units

            def emit_outproj_si(si):
                # output projection for one 128-row block of sq (fp16).
                # NOTE: row-splitting each K=128 pair-step into two
                # concurrent K=64 halves (to hide the per-step LDW)
                # HANGS the device — two row-tiled matmuls cannot
                # accumulate into the same PSUM bank concurrently.
                ot = osb.tile([P, D], f16, tag="ot", name="ot")
                for h in range(D // 512):
                    ps = gpps.tile([P, 512], f32, tag="gp", name="gp")
                    for p in range(NPAIR):
                        nc.tensor.matmul(
                            ps[:],
                            ctxt[p][:, si * P:(si + 1) * P],
                            wo3[:, p, h * 512:(h + 1) * 512],
                            start=(p == 0), stop=(p == NPAIR - 1))
                    nc.vector.tensor_copy(ot[:, h * 512:(h + 1) * 512],
                                          ps[:])
                nc.sync.dma_start(out.ap()[si * P:(si + 1) * P, :], ot[:])

            pending_si = []

            def normalize_chunk(jj, rinv, asb_all):
                # ctx^T[dk, sq] = av[0:64] * (1/av[64]) per head; one
                # broadcast matmul per pair covers both heads. Deferred
                # into the next chunk's compute so the den-DMA ->
                # reciprocal -> broadcast chain never stalls the PE.
                for p in range(NPAIR):
                    bc = gpps.tile([P, 512], f32, tag="gp", name="bc")
                    nc.tensor.matmul(bc[:],
                                     selh2_t[:, 128 * p:128 * (p + 1)],
                                     rinv[:], start=True, stop=True)
                    for e in range(2):
                        nc.vector.tensor_mul(
                            ctxt[p][64 * e:64 * e + 64,
                                    jj * 512:(jj + 1) * 512],
                            asb_all[(p, e)][0:64, :],
                            bc[64 * e:64 * e + 64, :])
                pending_si.extend(range(4 * jj, 4 * jj + 4))

            prev_chunk = None
            # HAM warmup: dummy matmuls keep the PE busy from ~6.5us
            # while the startup DMAs land — the real startup stream is
            # wire-bound and too sparse to hold the clock at 2.4 GHz,
            # so the bridge runs long enough to hand over warm
            nc.vector.memset(dummy_w[:], 0.0)
            dps = gpps.tile([P, 512], f32, tag="gp", name="gp")
            for r in range(4):
                nc.tensor.matmul(dps[:], dummy_w[:, 0:P], dummy_w[:],
                                 start=(r == 0), stop=(r == 3))
            # chunk-0 x loads + remaining pair weights in consumption
            # order (pair-0 weights already in flight above)
            xq0 = stage_x8(xq, 0, "xq")
            xk0 = stage_x8(xk, 0, "xk")
            xv0 = stage_xv(0)
            load_w_pair(nc.scalar, wq8m, wq, 1)
            load_w_pair(nc.scalar, wk8m, wk, 1)
            load_w_pair(nc.sync, wq8m, wq, 2)
            load_w_pair(nc.sync, wk8m, wk, 2)
            load_w_pair(nc.scalar, wq8m, wq, 3)
            load_w_pair(nc.scalar, wk8m, wk, 3)
            nc.sync.dma_start(wo3[:], wo_v)
            st0 = (xq0, xk0, xv0)
            # chunk 0: only pair-0's q/k upfront; the rest drips into the
            # attention loop (2 units/block) so the exp stream starts asap
            proj_q = project_units(0, st0)
            for _ in range(2):
                proj_q.pop(0)()
            for j in range(SJ):
                drip = 3 if j == 0 else 2
                if j + 1 < SJ:
                    proj_q = proj_q + project_units(j + 1, stage(j + 1))
                n_m = 4 * (j + 1)
                asb_all = {}
                last = j == SJ - 1
                # the last chunk's den rows ride the then-idle ACT queue
                # (they gate the tail normalize+outproj chain)
                dq = nc.scalar if last else nc.sync
                densb = rcp.tile([8, 512], f16, tag="den", name="den")
                for p in range(NPAIR):
                    if p == 2 and prev_chunk is not None:
                        normalize_chunk(*prev_chunk)
                        prev_chunk = None
                    if len(pending_si) > 2:
                        emit_outproj_si(pending_si.pop(0))
                    av = [avps.tile([65, 512], f32, tag="av", name="av")
                          for _ in range(2)]
                    pend = []  # (m, exp_tile, c0) awaiting their AV matmuls
                    for m in range(n_m):
                        # causal diagonal block d: columns [0, 128d) of
                        # this sq chunk are fully masked -> compute only
                        # the suffix [c0, 512)
                        d = m - 4 * j
                        c0 = 128 * d if d > 0 else 0
                        # ALL interruptions (AV pops, projection drips)
                        # land on even m-blocks, so odd blocks' scores
                        # ride back-to-back with the previous block's —
                        # same-shape matmul runs pipeline to ~0 cost
                        if m % 2 == 0:
                            if len(pend) > 3:
                                for _ in range(2):
                                    pm, pex, pc0 = pend.pop(0)
                                    for e in range(2):
                                        nc.tensor.matmul(
                                            av[e][:, pc0:512],
                                            vt[pm][:, 65 * (2 * p + e):
                                                   65 * (2 * p + e) + 65],
                                            pex[:, e * 512 + pc0:
                                                (e + 1) * 512],
                                            start=(pm == 0),
                                            stop=(pm == n_m - 1))
                            for _ in range(min(drip, len(proj_q))):
                                proj_q.pop(0)()
                        sc = scps.tile([P, 1024], f32, tag="sc", name="sc")
                        sc3 = sc[:].rearrange("p (e c) -> p e c", e=2)
                        for e in range(2):
                            nc.tensor.matmul(
                                sc[:, e * 512 + c0:(e + 1) * 512],
                                kTt[p][64 * e:64 * e + 64,
                                       m * P:(m + 1) * P],
                                qTt[p][64 * e:64 * e + 64,
                                       j * 512 + c0:(j + 1) * 512],
                                start=True, stop=True)
                        ex = expp.tile([P, 1024], f16, tag="ex", name="ex")
                        ex3 = ex[:].rearrange("p (e c) -> p e c", e=2)
                        nc.scalar.activation(ex3[:, :, c0:512],
                                             sc3[:, :, c0:512], Exp,
                                             scale=0.125 * ISCALE)
                        if d >= 0:
                            # zero the masked triangle of the diagonal
                            # window (unmasked logits are O(2), so exp
                            # stayed finite); e-halves split across
                            # GpSimd and Vector so neither engine's
                            # serial mul stream paces the AV drain
                            nc.gpsimd.tensor_mul(
                                ex3[:, 0:1, c0:c0 + P],
                                ex3[:, 0:1, c0:c0 + P],
                                mask01_t[:][:, None, :])
                            nc.vector.tensor_mul(
                                ex3[:, 1:2, c0:c0 + P],
                                ex3[:, 1:2, c0:c0 + P],
                                mask01_t[:][:, None, :])
                        pend.append((m, ex, c0))
                    for pm, pex, pc0 in pend:
                        for e in range(2):
                            nc.tensor.matmul(
                                av[e][:, pc0:512],
                                vt[pm][:, 65 * (2 * p + e):
                                       65 * (2 * p + e) + 65],
                                pex[:, e * 512 + pc0:(e + 1) * 512],
                                start=(pm == 0), stop=(pm == n_m - 1))
                    # stage av in SBUF; DMA its denominator row (partition
                    # 64) into a per-pair [2, 512] tile, then reciprocal
                    # and GpSimd partition-replication — all tiny and off
                    # the PE's critical path
                    for e in range(2):
                        asb = avsb.tile([65, 512], f16, tag="asb",
                                        name="asb")
                        nc.vector.tensor_copy(asb[:], av[e][:])
                        asb_all[(p, e)] = asb
                        dq.dma_start(densb[2 * p + e:2 * p + e + 1, :],
                                     asb[64:65, :])
                # issue the reciprocal eagerly so rinv is long done when
                # the deferred broadcast consumes it next chunk; the fast
                # approx (~18 bits, ~5x quicker) needs fp32 in/out
                den32 = rcp.tile([8, 512], f32, tag="dn32", name="dn32")
                rinv32 = rcp.tile([8, 512], f32, tag="rv32", name="rv32")
                rinv8 = rcp.tile([8, 512], f16, tag="rinv", name="rinv")
                nc.vector.tensor_copy(den32[:], densb[:])
                nc.vector.reciprocal_approx_fast(rinv32[:], den32[:])
                nc.vector.tensor_copy(rinv8[:], rinv32[:])
                prev_chunk = (j, rinv8, asb_all)

            # drain the previous chunk's ready outprojs BEFORE the final
            # normalize: its broadcast matmuls wait ~5us on the den ->
            # recip chain and would block these in the PE's in-order
            # queue
            while pending_si:
                emit_outproj_si(pending_si.pop(0))
            normalize_chunk(*prev_chunk)
            while pending_si:
                emit_outproj_si(pending_si.pop(0))

    nc.compile()
    return nc


def _get_nc(s_len):
    if s_len not in _BUILD_CACHE:
        _BUILD_CACHE[s_len] = _build(s_len)
    return _BUILD_CACHE[s_len]


def kernel(query, key, value, mask, Wq, bq, Wk, bk, Wv, bv, Wo, bo):
    import ml_dtypes
    from concourse.bass_utils import run_bass_kernel_spmd

    E4 = ml_dtypes.float8_e4m3
    query = np.asarray(query, dtype=np.float32)
    key = np.asarray(key, dtype=np.float32)
    value = np.asarray(value, dtype=np.float32)
    mask = np.asarray(mask, dtype=np.float32)
    Wq, Wk, Wv, Wo = (np.asarray(w, dtype=np.float32) for w in (Wq, Wk, Wv, Wo))
    bq, bk, bv, bo = (np.asarray(b, dtype=np.float32) for b in (bq, bk, bv, bo))

    b_sz, s_len, d = query.shape
    m2 = mask.reshape(s_len, s_len)
    causal = bool(
        np.array_equal(m2, np.triu(np.ones((s_len, s_len), np.float32), k=1)))
    zero_bias = not (bq.any() or bk.any() or bv.any())
    if not (causal and zero_bias):
        raise NotImplementedError("fp8 kernel supports causal zero-bias only")

    nc = _get_nc(s_len)

    f16 = np.float16
    pp_ = np.arange(P)[:, None]
    cc_ = np.arange(P)[None, :]
    mask01 = np.where(pp_ > cc_, 0.0, 1.0).astype(f16)
    selh2 = np.zeros((8, 4 * 128), f16)
    for p in range(4):
        for e in range(2):
            selh2[2 * p + e, 128 * p + 64 * e:128 * p + 64 * e + 64] = 1.0

    def shuf_w(w):
        # [(d t p), (i m)] -> [(i p), (d t m)] so each pair's weights
        # are one contiguous row-block in DRAM
        w5 = w.reshape(4, 2, P, 4, P)          # d t p i m
        return np.ascontiguousarray(
            w5.transpose(3, 2, 0, 1, 4).reshape(4 * P, 4 * 2 * P))

    in_maps = []
    for c in range(N_CORES):
        b = c // 2
        g = c % 2
        cols = slice(GD * g, GD * g + GD)
        im = {
            "xq": np.ascontiguousarray(query[b].T).astype(E4),
            "xk": np.ascontiguousarray(key[b].T).astype(E4),
            "xv": np.ascontiguousarray(value[b].T).astype(f16),
            "wq": shuf_w(Wq[:, cols] * WSCALE).astype(E4),
            "wk": shuf_w(Wk[:, cols] * WSCALE).astype(E4),
            "wv": np.ascontiguousarray(Wv[:, cols]).astype(f16),
            "wo": np.ascontiguousarray(Wo[cols, :]).astype(f16),
            "mask01": mask01,
            "selh2": selh2,
        }
        in_maps.append(im)

    res = run_bass_kernel_spmd(nc, in_maps, list(range(N_CORES)))

    out = np.empty((b_sz, s_len, d), np.float32)
    for b in range(b_sz):
        out[b] = (res.results[2 * b]["out"].astype(np.float32)
                  + res.results[2 * b + 1]["out"].astype(np.float32) + bo)
    return out
